# revision 1
# baseline (speedup 1.0000x reference)
"""DA-HGNN forward kernel, row-sharded SPMD across 8 Trainium2 NeuronCores.

Self-contained: takes full inputs, shards host-side, runs one Bass/Tile
program on cores 0-7 with collectives, returns the full [4096, 256] output.
"""
import numpy as np

from contextlib import ExitStack

from concourse import bass, mybir, bacc, tile
from concourse.bass_utils import run_bass_kernel_spmd

f32 = mybir.dt.float32
bf16 = mybir.dt.bfloat16
u32 = mybir.dt.uint32
AF = mybir.ActivationFunctionType
OP = mybir.AluOpType
AX = mybir.AxisListType

N = 4096          # nodes == hyperedges
F = 784           # input features
D = 256           # hidden dim
NCORE = 8
SH = N // NCORE   # 512 rows per core
KCH = 112         # 7 chunks of 112 over F
NKF = 7
TOPK = 11
SIGMA = 0.3
SLOPE = 0.2
DV = float(np.float32(1.0) / np.sqrt(np.float32(TOPK)))
NEG_BIG = -3.0e38

JT = 512          # j-tile width for rho phase (= shard width)
NJT = N // JT     # 8
JW = 256          # phase-A stream slab width


def _build():
    nc = bacc.Bacc("TRN2", target_bir_lowering=False, debug=False,
                   num_devices=NCORE)

    # ---- I/O -------------------------------------------------------------
    xt_in = nc.dram_tensor("xt", [NKF, KCH, N], f32, kind="ExternalInput")
    xtc_in = nc.dram_tensor("xtc", [NKF, KCH, SH], f32, kind="ExternalInput")
    th_in = nc.dram_tensor("theta", [NKF, KCH, D], f32, kind="ExternalInput")
    w_in = nc.dram_tensor("w", [2, 128, D], f32, kind="ExternalInput")
    al_in = nc.dram_tensor("alpha", [2, 2 * D], f32, kind="ExternalInput")
    out_t = nc.dram_tensor("out", [N, D], f32, kind="ExternalOutput")

    # ---- internal DRAM (collective bounces) ------------------------------
    sq_in = nc.dram_tensor("sq_in", [1, SH], f32)
    sq_ag = nc.dram_tensor("sq_ag", [1, N], f32, addr_space="Shared")
    hbt_in = nc.dram_tensor("hbt_in", [N, SH], bf16)
    ht_ag = nc.dram_tensor("ht_ag", [N * NCORE, SH], bf16, addr_space="Shared")
    s1_in = nc.dram_tensor("s1_in", [N, D + 1], f32)
    s1_rs = nc.dram_tensor("s1_rs", [SH, D + 1], f32)
    s1_full = nc.dram_tensor("s1_full", [N, D + 1], f32, addr_space="Shared")
    agv_in = nc.dram_tensor("agv_in", [SH, 5], f32)
    agv_out = nc.dram_tensor("agv_out", [N, 5], f32, addr_space="Shared")
    re_in = nc.dram_tensor("re_in", [N, 1], f32)
    re_rs = nc.dram_tensor("re_rs", [SH, 1], f32)
    mx_in = nc.dram_tensor("mx_in", [1, 8], f32)
    mx_out = nc.dram_tensor("mx_out", [1, 8], f32, addr_space="Shared")
    dnx_in = nc.dram_tensor("dnx_in", [N, 1], f32)
    dnx_rs = nc.dram_tensor("dnx_rs", [SH, 1], f32)
    dne_in = nc.dram_tensor("dne_in", [N, 1], f32)
    dne_ar = nc.dram_tensor("dne_ar", [N, 1], f32, addr_space="Shared")
    nx_in = nc.dram_tensor("nx_in", [N, D], f32)
    nx_rs = nc.dram_tensor("nx_rs", [SH, D], f32)
    ne_in = nc.dram_tensor("ne_in", [N, D], f32)
    ne_ar = nc.dram_tensor("ne_ar", [N, D], f32, addr_space="Shared")
    wv_dram = nc.dram_tensor("wv_dram", [2, D], f32)

    RG = [list(range(NCORE))]

    with tile.TileContext(nc) as tc, ExitStack() as top:
        cp = top.enter_context(tc.tile_pool(name="const", bufs=1))
        sm = top.enter_context(tc.tile_pool(name="smalls", bufs=2))
        tp = top.enter_context(tc.tile_pool(name="tmps", bufs=3))
        rp = top.enter_context(tc.tile_pool(name="rows", bufs=1))

        def rsqrt_(out_ap, in_ap, scale, shape):
            t_ = sm.tile(shape, f32, tag="rsqt", name="rsqt")
            nc.scalar.activation(out=t_[:], in_=in_ap, func=AF.Sqrt, scale=scale)
            nc.vector.reciprocal(out=out_ap, in_=t_[:])

        # constants
        ident = cp.tile([128, 128], f32, tag="ident", name="ident")
        ident_b = cp.tile([128, 128], bf16, tag="identb", name="identb")
        ones_col = cp.tile([128, 1], f32, tag="ones", name="ones")
        nc.vector.memset(ones_col[:], 1.0)
        ones8 = cp.tile([128, 8], f32, tag="ones8", name="ones8")
        nc.vector.memset(ones8[:], 1.0)

        w_sb = [cp.tile([128, D], f32, tag=f"w{k}", name=f"w{k}") for k in range(2)]
        for k in range(2):
            nc.sync.dma_start(w_sb[k][:], w_in[k, :, :])

        # long-lived big tensors
        es_hf = ExitStack()
        hfp = es_hf.enter_context(tc.tile_pool(name="hfinal", bufs=4))
        hf = [hfp.tile([128, N], bf16, tag="hf", name="hf") for _ in range(4)]
        es_xtc = ExitStack()
        xp = es_xtc.enter_context(tc.tile_pool(name="xtc", bufs=1))
        xtc = [xp.tile([KCH + (1 if k == 0 else 0), SH], f32, tag=f"xtc{k}", name=f"xtc{k}")
               for k in range(NKF)]
        nc.vector.memset(xtc[0][:, :], 1.0)  # row 112 stays ones
        for k in range(NKF):
            nc.sync.dma_start(xtc[k][0:KCH, :], xtc_in[k, :, :])

        # =================================================================
        # PHASE A: scores = Xc @ X.T - sq/2 ; top-k -> H (bf16) ; H^T ; AG
        # =================================================================
        es_a = ExitStack()
        ap_ = es_a.enter_context(tc.tile_pool(name="aphase", bufs=1))
        ap2 = es_a.enter_context(tc.tile_pool(name="aphase2", bufs=2))
        ppA = es_a.enter_context(tc.tile_pool(name="ppA", bufs=3, space="PSUM"))
        ppT = es_a.enter_context(tc.tile_pool(name="ppTa", bufs=2, space="PSUM"))
        ppq = es_a.enter_context(tc.tile_pool(name="ppq", bufs=1, space="PSUM"))

        io128 = ap2.tile([128, 128], f32, tag="io128", name="io128", bufs=1)
        nc.gpsimd.iota(io128[:], pattern=[[1, 128]], base=0, channel_multiplier=-1,
                       allow_small_or_imprecise_dtypes=True)
        nc.vector.tensor_scalar(out=ident[:], in0=io128[:], scalar1=0.0,
                                scalar2=None, op0=OP.is_equal)
        nc.vector.tensor_copy(out=ident_b[:], in_=ident[:])

        # sq_c = colsum(XTc^2)  -> [1, 512] -> AG -> nsqh row [1, 4096]
        ps_sq = ppq.tile([1, SH], f32, tag="ps_sq", name="ps_sq")
        for k in range(NKF):
            sqt = ap2.tile([KCH, SH], f32, tag="sqsq", name="sqsq", bufs=1)
            nc.scalar.activation(out=sqt[:], in_=xtc[k][0:KCH, :], func=AF.Square)
            nc.tensor.matmul(ps_sq[:], ones_col[0:KCH, :], sqt[:],
                             start=(k == 0), stop=(k == NKF - 1))
        sq_row = rp.tile([1, SH], f32, tag="rowsm", name="sqrow")
        nc.scalar.copy(out=sq_row[:], in_=ps_sq[:])
        nc.sync.dma_start(sq_in[:, :], sq_row[:])
        nc.gpsimd.collective_compute("AllGather", OP.bypass, replica_groups=RG,
                                     ins=[sq_in.ap()], outs=[sq_ag.ap()])
        nsqh_row = ap_.tile([1, N], f32, tag="nsqh", name="nsqh")
        nc.sync.dma_start(nsqh_row[:], sq_ag[:, :])
        nc.scalar.activation(out=nsqh_row[:], in_=nsqh_row[:], func=AF.Copy,
                             scale=-0.5)

        iota5 = ap_.tile([128, 512], f32, tag="iota5", name="iota5")
        nc.gpsimd.iota(iota5[:], pattern=[[1, 512]], base=0, channel_multiplier=0,
                       allow_small_or_imprecise_dtypes=True)

        scores = [ap_.tile([128, N], f32, tag=f"sc{i}", name=f"sc{i}") for i in range(2)]

        for half in range(2):
            for j in range(N // JW):
                jsl = slice(j * JW, (j + 1) * JW)
                slab = [ap2.tile([KCH + (1 if k == 0 else 0), JW], f32,
                                 tag=f"slab{k}", name=f"slab{k}") for k in range(NKF)]
                for k in range(NKF):
                    nc.sync.dma_start(slab[k][0:KCH, :], xt_in[k, :, jsl])
                nc.sync.dma_start(slab[0][KCH:KCH + 1, :],
                                  nsqh_row[:, jsl])
                for ii in range(2):
                    i = 2 * half + ii
                    ps = ppA.tile([128, JW], f32, tag="psA", name="psA")
                    for k in range(NKF):
                        kk = KCH + (1 if k == 0 else 0)
                        nc.tensor.matmul(ps[:],
                                         xtc[k][0:kk, i * 128:(i + 1) * 128],
                                         slab[k][0:kk, :],
                                         start=(k == 0), stop=(k == NKF - 1))
                    nc.scalar.copy(out=scores[ii][:, jsl], in_=ps[:])

            # top-k threshold + exact tie-break -> H rows (bf16 0/1)
            for ii in range(2):
                i = 2 * half + ii
                m1 = sm.tile([128, 8], f32, tag="m1", name="m1")
                m2 = sm.tile([128, 8], f32, tag="m2", name="m2")
                tmpf = ap2.tile([128, N], f32, tag="tmpf", name="tmpf", bufs=1)
                nc.vector.max(m1[:], scores[ii][:])
                nc.vector.match_replace(tmpf[:], m1[:], scores[ii][:], NEG_BIG)
                nc.vector.max(m2[:], tmpf[:])
                tq = m2[:, 2:3]  # 11th largest
                hA = ap2.tile([128, N], bf16, tag="hwork", name="hwork")
                nc.vector.tensor_scalar(out=hA[:], in0=scores[ii][:], scalar1=tq,
                                        scalar2=None, op0=OP.is_gt)
                cst = sm.tile([128, 1], f32, tag="cst", name="cst")
                nc.vector.reduce_sum(cst[:], hA[:], axis=AX.X)
                need = sm.tile([128, 1], f32, tag="need", name="need")
                nc.vector.tensor_scalar(out=need[:], in0=cst[:], scalar1=-1.0,
                                        scalar2=float(TOPK), op0=OP.mult,
                                        op1=OP.add)
                t8 = sm.tile([128, 8], f32, tag="t8", name="t8")
                nc.vector.tensor_scalar(out=t8[:], in0=ones8[:], scalar1=tq,
                                        scalar2=None, op0=OP.mult)
                idx8 = sm.tile([128, 8], u32, tag="idx8", name="idx8")
                nc.vector.max_index(idx8[:], t8[:], scores[ii][:])
                idxf = sm.tile([128, 8], f32, tag="idxf", name="idxf")
                nc.vector.tensor_copy(out=idxf[:], in_=idx8[:])
                gate1 = sm.tile([128, 1], f32, tag="gate1", name="gate1")
                nc.vector.tensor_scalar(out=gate1[:], in0=need[:], scalar1=1.5,
                                        scalar2=None, op0=OP.is_gt)
                gm1 = sm.tile([128, 1], f32, tag="gm1", name="gm1")
                nc.vector.tensor_scalar(out=gm1[:], in0=gate1[:], scalar1=-1.0,
                                        scalar2=None, op0=OP.add)
                idx1g = sm.tile([128, 1], f32, tag="idx1g", name="idx1g")
                nc.vector.scalar_tensor_tensor(out=idx1g[:], in0=idxf[:, 1:2],
                                               scalar=gate1[:], in1=gm1[:],
                                               op0=OP.mult, op1=OP.add)
                hB = ap2.tile([128, N], bf16, tag="hwork", name="hwork")
                for tb in range(8):
                    tsl = slice(tb * 512, (tb + 1) * 512)
                    i0a = sm.tile([128, 1], f32, tag="i0a", name="i0a")
                    nc.vector.tensor_scalar(out=i0a[:], in0=idxf[:, 0:1],
                                            scalar1=float(-tb * 512),
                                            scalar2=None, op0=OP.add)
                    i1a = sm.tile([128, 1], f32, tag="i1a", name="i1a")
                    nc.vector.tensor_scalar(out=i1a[:], in0=idx1g[:],
                                            scalar1=float(-tb * 512),
                                            scalar2=None, op0=OP.add)
                    nc.vector.scalar_tensor_tensor(out=hB[:, tsl], in0=iota5[:],
                                                   scalar=i0a[:], in1=hA[:, tsl],
                                                   op0=OP.is_equal, op1=OP.add)
                    nc.vector.scalar_tensor_tensor(out=hf[i][:, tsl],
                                                   in0=iota5[:], scalar=i1a[:],
                                                   in1=hB[:, tsl],
                                                   op0=OP.is_equal, op1=OP.add)
                for ec in range(32):
                    pt = ppT.tile([128, 128], bf16, tag="ptp", name="ptp")
                    nc.tensor.transpose(pt[:], hf[i][:, ec * 128:(ec + 1) * 128],
                                        ident_b[:])
                    hev = ap2.tile([128, 128], bf16, tag="hbtev", name="hbtev",
                                   bufs=3)
                    nc.scalar.copy(out=hev[:], in_=pt[:])
                    nc.sync.dma_start(
                        hbt_in[ec * 128:(ec + 1) * 128,
                               i * 128:(i + 1) * 128], hev[:])

        nc.gpsimd.collective_compute("AllGather", OP.bypass, replica_groups=RG,
                                     ins=[hbt_in.ap()], outs=[ht_ag.ap()])
        es_a.close()

        # =================================================================
        # PHASE B: Y = Xc @ theta (+ones col);  S1 = H^T @ [Y|1] -> RS + AG
        # =================================================================
        es_b = ExitStack()
        bp = es_b.enter_context(tc.tile_pool(name="bphase", bufs=1))
        bp2 = es_b.enter_context(tc.tile_pool(name="bphase2", bufs=3))
        ppB = es_b.enter_context(tc.tile_pool(name="ppB", bufs=2, space="PSUM"))

        thsb = [bp.tile([KCH, D], f32, tag=f"th{k}", name=f"th{k}") for k in range(NKF)]
        for k in range(NKF):
            nc.sync.dma_start(thsb[k][:], th_in[k, :, :])
        yplus = [bp.tile([128, D + 1], f32, tag=f"yp{i}", name=f"yp{i}") for i in range(4)]
        for i in range(4):
            ps = ppB.tile([128, D], f32, tag="psY", name="psY")
            for k in range(NKF):
                nc.tensor.matmul(ps[:],
                                 xtc[k][0:KCH, :][:, i * 128:(i + 1) * 128],
                                 thsb[k][:], start=(k == 0), stop=(k == NKF - 1))
            nc.scalar.copy(out=yplus[i][:, 0:D], in_=ps[:])
            nc.vector.memset(yplus[i][:, D:D + 1], 1.0)

        # S1[e, :] = sum_i H[i,e] * Yplus[i, :]  (lhsT = upconverted H chunk)
        for m in range(32):
            ps = ppB.tile([128, D + 1], f32, tag="psS1", name="psS1")
            for i in range(4):
                up = bp2.tile([128, 128], f32, tag="hup", name="hup")
                nc.vector.tensor_copy(out=up[:],
                                      in_=hf[i][:, m * 128:(m + 1) * 128])
                nc.tensor.matmul(ps[:], up[:], yplus[i][:, :],
                                 start=(i == 0), stop=(i == 3))
            s1t = bp2.tile([128, D + 1], f32, tag="s1ev", name="s1ev")
            nc.scalar.copy(out=s1t[:], in_=ps[:])
            nc.sync.dma_start(s1_in[m * 128:(m + 1) * 128, :], s1t[:])
        nc.gpsimd.collective_compute("ReduceScatter", OP.add, replica_groups=RG,
                                     ins=[s1_in.ap()], outs=[s1_rs.ap()])
        nc.gpsimd.collective_compute("AllGather", OP.bypass, replica_groups=RG,
                                     ins=[s1_rs.ap()], outs=[s1_full.ap()])
        es_b.close()
        es_xtc.close()

        # =================================================================
        # PHASE C1: my Xl slab; XlcT; XlW; u_x, v_e; sigma*n, 1/n, diag
        # =================================================================
        es_c = ExitStack()
        cpl = es_c.enter_context(tc.tile_pool(name="cphase", bufs=1))
        es_hbt = ExitStack()
        hbtp = es_hbt.enter_context(tc.tile_pool(name="hbt", bufs=1))
        es_xht = ExitStack()
        xhp = es_xht.enter_context(tc.tile_pool(name="xht", bufs=1))
        es_ppc = ExitStack()
        ppC = es_ppc.enter_context(tc.tile_pool(name="ppC", bufs=2, space="PSUM"))
        es_tp = ExitStack()
        ppTf = es_tp.enter_context(tc.tile_pool(name="ppTf", bufs=2, space="PSUM"))

        al_x = cpl.tile([1, 2 * D], f32, tag="alx", name="alx")
        nc.sync.dma_start(al_x[:], al_in[0:1, :])
        al_e = cpl.tile([1, 2 * D], f32, tag="ale", name="ale")
        nc.sync.dma_start(al_e[:], al_in[1:2, :])
        xlc = [cpl.tile([128, D], f32, tag=f"xlc{i}", name=f"xlc{i}") for i in range(4)]
        sgn = [sm.tile([128, 1], f32, tag=f"sgn{i}", name=f"sgn{i}") for i in range(4)]
        rcn = [sm.tile([128, 1], f32, tag=f"rcn{i}", name=f"rcn{i}") for i in range(4)]
        diag = [sm.tile([128, 1], f32, tag=f"diag{i}", name=f"diag{i}") for i in range(4)]
        for i in range(4):
            sl = tp.tile([128, D + 1], f32, tag="slabs1", name="slabs1")
            nc.sync.dma_start(sl[:], s1_rs[i * 128:(i + 1) * 128, :])
            dde = sm.tile([128, 1], f32, tag="dde", name="dde")
            rsqrt_(dde[:], sl[:, D:D + 1], float(TOPK), [128, 1])
            nc.vector.tensor_scalar(out=xlc[i][:], in0=sl[:, 0:D],
                                    scalar1=dde[:], scalar2=None, op0=OP.mult)
            nsq = sm.tile([128, 1], f32, tag="nsq", name="nsq")
            tr = tp.tile([128, D], f32, tag="t256", name="trsq", bufs=6)
            nc.scalar.activation(out=tr[:], in_=xlc[i][:], func=AF.Square,
                                 accum_out=nsq[:])
            nc.scalar.activation(out=sgn[i][:], in_=nsq[:], func=AF.Sqrt,
                                 scale=float(SIGMA) * float(SIGMA))
            rsqrt_(rcn[i][:], nsq[:], 1.0, [128, 1])
            xhc = tp.tile([128, D], f32, tag="t256", name="xhc", bufs=6)
            nc.vector.tensor_scalar(out=xhc[:], in0=xlc[i][:], scalar1=rcn[i][:],
                                    scalar2=None, op0=OP.mult)
            tr2 = tp.tile([128, D], f32, tag="t256", name="trsq", bufs=6)
            nc.vector.scalar_tensor_tensor(out=tr2[:], in0=xlc[i][:], scalar=1.0,
                                           in1=xhc[:], op0=OP.mult, op1=OP.mult,
                                           accum_out=diag[i][:])

        xlct = [cpl.tile([128, SH], f32, tag=f"xlct{d}", name=f"xlct{d}") for d in range(2)]
        for i in range(4):
            for d in range(2):
                pt = ppTf.tile([128, 128], f32, tag="ptpf", name="ptpf")
                nc.tensor.transpose(pt[:], xlc[i][:, d * 128:(d + 1) * 128],
                                    ident[:])
                nc.scalar.copy(out=xlct[d][:, i * 128:(i + 1) * 128], in_=pt[:])

        xlw = [cpl.tile([128, D], f32, tag=f"xlw{i}", name=f"xlw{i}") for i in range(4)]
        ax1b = cpl.tile([128, D], f32, tag="ax1b", name="ax1b")
        nc.gpsimd.partition_broadcast(ax1b[:], al_x[:, 0:D])
        ae2b = cpl.tile([128, D], f32, tag="ae2b", name="ae2b")
        nc.gpsimd.partition_broadcast(ae2b[:], al_e[:, D:2 * D])
        u_x = [sm.tile([128, 1], f32, tag=f"ux{i}", name=f"ux{i}") for i in range(4)]
        v_e = [sm.tile([128, 1], f32, tag=f"ve{i}", name=f"ve{i}") for i in range(4)]
        for i in range(4):
            ps = ppC.tile([128, D], f32, tag="psXW", name="psXW")
            for k in range(2):
                nc.tensor.matmul(ps[:], xlct[k][:, i * 128:(i + 1) * 128],
                                 w_sb[k][:], start=(k == 0), stop=(k == 1))
            nc.scalar.copy(out=xlw[i][:], in_=ps[:])
            t1 = tp.tile([128, D], f32, tag="t256", name="uvtmp", bufs=6)
            nc.vector.scalar_tensor_tensor(out=t1[:], in0=xlw[i][:], scalar=1.0,
                                           in1=ax1b[:], op0=OP.mult, op1=OP.mult,
                                           accum_out=u_x[i][:])
            t2 = tp.tile([128, D], f32, tag="t256", name="uvtmp", bufs=6)
            nc.vector.scalar_tensor_tensor(out=t2[:], in0=xlw[i][:], scalar=1.0,
                                           in1=ae2b[:], op0=OP.mult, op1=OP.mult,
                                           accum_out=v_e[i][:])

        # =================================================================
        # PHASE C2: full pass -> X^lT (G rhs), Z;  E = dv*H_c@Z; u_e, v_x
        # =================================================================
        hbt = hbtp.tile([128, 32, SH], bf16, tag="hbt", name="hbt")
        nc.sync.dma_start(hbt[:],
                          hbt_in.ap().rearrange("(ec p) i -> p ec i", p=128))
        xht = [xhp.tile([128, N], f32, tag=f"xht{d}", name=f"xht{d}") for d in range(2)]
        es_z = ExitStack()
        zp = es_z.enter_context(tc.tile_pool(name="zp", bufs=1))
        z = zp.tile([128, 32, D], bf16, tag="z", name="z")
        for m in range(32):
            sl = tp.tile([128, D + 1], f32, tag="slabs1", name="slabs1")
            nc.sync.dma_start(sl[:], s1_full[m * 128:(m + 1) * 128, :])
            dde = sm.tile([128, 1], f32, tag="dde", name="dde")
            rsqrt_(dde[:], sl[:, D:D + 1], float(TOPK), [128, 1])
            de1 = sm.tile([128, 1], f32, tag="de1", name="de1")
            rsqrt_(de1[:], sl[:, D:D + 1], 1.0, [128, 1])
            xlm = tp.tile([128, D], f32, tag="t256", name="xlm", bufs=6)
            nc.vector.tensor_scalar(out=xlm[:], in0=sl[:, 0:D], scalar1=dde[:],
                                    scalar2=None, op0=OP.mult)
            nc.vector.tensor_scalar(out=z[:, m, :], in0=xlm[:], scalar1=de1[:],
                                    scalar2=None, op0=OP.mult)
            nsq = sm.tile([128, 1], f32, tag="nsq", name="nsq")
            tr = tp.tile([128, D], f32, tag="t256", name="trsq", bufs=6)
            nc.scalar.activation(out=tr[:], in_=xlm[:], func=AF.Square,
                                 accum_out=nsq[:])
            rc = sm.tile([128, 1], f32, tag="rcm", name="rcm")
            rsqrt_(rc[:], nsq[:], 1.0, [128, 1])
            xhm = tp.tile([128, D], f32, tag="t256", name="xhm", bufs=6)
            nc.vector.tensor_scalar(out=xhm[:], in0=xlm[:], scalar1=rc[:],
                                    scalar2=None, op0=OP.mult)
            for d in range(2):
                pt = ppTf.tile([128, 128], f32, tag="ptpf", name="ptpf")
                nc.tensor.transpose(pt[:], xhm[:, d * 128:(d + 1) * 128], ident[:])
                nc.scalar.copy(out=xht[d][:, m * 128:(m + 1) * 128], in_=pt[:])

        e_c = [cpl.tile([128, D], f32, tag=f"ec{i}", name=f"ec{i}") for i in range(4)]
        for i in range(4):
            ps = ppC.tile([128, D], f32, tag="psE", name="psE")
            for ec in range(32):
                nc.tensor.matmul(ps[:], hbt[:, ec, i * 128:(i + 1) * 128],
                                 z[:, ec, :], start=(ec == 0), stop=(ec == 31))
            nc.scalar.activation(out=e_c[i][:], in_=ps[:], func=AF.Copy, scale=DV)
        es_z.close()
        es_tp.close()
        es_ppc.close()

        # w-vectors: wx2 = W @ alpha_x[256:], we1 = W @ alpha_e[:256]
        ax2b = cpl.tile([128, D], f32, tag="ax2b", name="ax2b")
        nc.gpsimd.partition_broadcast(ax2b[:], al_x[:, D:2 * D])
        ae1b = cpl.tile([128, D], f32, tag="ae1b", name="ae1b")
        nc.gpsimd.partition_broadcast(ae1b[:], al_e[:, 0:D])
        for vi, ab in enumerate([ax2b, ae1b]):
            for k in range(2):
                wvp = sm.tile([128, 1], f32, tag="wvp", name="wvp")
                t = tp.tile([128, D], f32, tag="t256", name="uvtmp", bufs=6)
                nc.vector.scalar_tensor_tensor(out=t[:], in0=w_sb[k][:],
                                               scalar=1.0, in1=ab[:],
                                               op0=OP.mult, op1=OP.mult,
                                               accum_out=wvp[:])
                nc.sync.dma_start(wv_dram[vi:vi + 1, k * 128:(k + 1) * 128],
                                  wvp[:])
        wx2b = cpl.tile([128, D], f32, tag="wx2b", name="wx2b")
        we1b = cpl.tile([128, D], f32, tag="we1b", name="we1b")
        wvrow = rp.tile([1, D], f32, tag="rowsm", name="wvrow")
        nc.sync.dma_start(wvrow[:], wv_dram[0:1, :])
        nc.gpsimd.partition_broadcast(wx2b[:], wvrow[:])
        wvrow2 = rp.tile([1, D], f32, tag="rowsm", name="wvrow2")
        nc.sync.dma_start(wvrow2[:], wv_dram[1:2, :])
        nc.gpsimd.partition_broadcast(we1b[:], wvrow2[:])
        u_e = [sm.tile([128, 1], f32, tag=f"ue{i}", name=f"ue{i}") for i in range(4)]
        v_x = [sm.tile([128, 1], f32, tag=f"vx{i}", name=f"vx{i}") for i in range(4)]
        for i in range(4):
            t1 = tp.tile([128, D], f32, tag="t256", name="uvtmp", bufs=6)
            nc.vector.scalar_tensor_tensor(out=t1[:], in0=e_c[i][:], scalar=1.0,
                                           in1=we1b[:], op0=OP.mult, op1=OP.mult,
                                           accum_out=u_e[i][:])
            t2 = tp.tile([128, D], f32, tag="t256", name="uvtmp", bufs=6)
            nc.vector.scalar_tensor_tensor(out=t2[:], in0=e_c[i][:], scalar=1.0,
                                           in1=wx2b[:], op0=OP.mult, op1=OP.mult,
                                           accum_out=v_x[i][:])

        # =================================================================
        # PHASE C3: adjacency + masked cosine rho (streams ht_ag slabs)
        # =================================================================
        NJQ = 16
        rho_x = [sm.tile([128, 1], f32, tag=f"rho{i}", name=f"rho{i}") for i in range(4)]
        rho_cols = [cpl.tile([128, NJQ], f32, tag=f"rhoc{i}", name=f"rhoc{i}") for i in range(4)]
        es_slab = ExitStack()
        slp = es_slab.enter_context(tc.tile_pool(name="htslab", bufs=2))
        rtp = es_slab.enter_context(tc.tile_pool(name="rhotmp", bufs=2))
        ppR = es_slab.enter_context(tc.tile_pool(name="ppR", bufs=2, space="PSUM"))
        for jq in range(NJQ):
            r_, ch = jq // 2, jq % 2
            psAs = []
            for i in range(4):
                psA = ppR.tile([128, 256], f32, tag=f"psadj{i}",
                               name=f"psadj{i}", bufs=1)
                psAs.append(psA)
            for eh in range(2):
                hts = slp.tile([128, 16, 256], bf16, tag="hts", name="hts")
                nc.sync.dma_start(
                    hts[:], ht_ag[r_ * N + eh * 2048:r_ * N + (eh + 1) * 2048,
                                  ch * 256:(ch + 1) * 256]
                    .rearrange("(ec p) i -> p ec i", p=128))
                for i in range(4):
                    for ec in range(16):
                        nc.tensor.matmul(
                            psAs[i][:],
                            hbt[:, eh * 16 + ec, i * 128:(i + 1) * 128],
                            hts[:, ec, :], start=(eh == 0 and ec == 0),
                            stop=(eh == 1 and ec == 15))
            for i in range(4):
                psG = ppR.tile([128, 256], f32, tag="psg", name="psg")
                for k in range(2):
                    nc.tensor.matmul(psG[:], xlct[k][:, i * 128:(i + 1) * 128],
                                     xht[k][:, jq * 256:(jq + 1) * 256],
                                     start=(k == 0), stop=(k == 1))
                g_sb = rtp.tile([128, 256], f32, tag="gsb", name="gsb", bufs=1)
                nc.scalar.copy(out=g_sb[:], in_=psG[:])
                t1 = rtp.tile([128, 256], f32, tag="rt1", name="rt1", bufs=1)
                nc.vector.scalar_tensor_tensor(out=t1[:], in0=psAs[i][:],
                                               scalar=0.5, in1=g_sb[:],
                                               op0=OP.is_gt, op1=OP.mult)
                t2 = rtp.tile([128, 256], f32, tag="rt2", name="rt2", bufs=1)
                nc.vector.scalar_tensor_tensor(out=t2[:], in0=g_sb[:],
                                               scalar=sgn[i][:], in1=t1[:],
                                               op0=OP.is_gt, op1=OP.mult,
                                               accum_out=rho_cols[i][:, jq:jq + 1])
        es_slab.close()
        es_xht.close()
        es_hbt.close()
        for i in range(4):
            r1 = sm.tile([128, 1], f32, tag="r1", name="r1")
            nc.vector.reduce_sum(r1[:], rho_cols[i][:], axis=AX.X)
            r2 = sm.tile([128, 1], f32, tag="r2", name="r2")
            nc.vector.tensor_tensor(out=r2[:], in0=r1[:], in1=diag[i][:],
                                    op=OP.subtract)
            nc.vector.tensor_tensor(out=rho_x[i][:], in0=r2[:], in1=rcn[i][:],
                                    op=OP.mult)

        # =================================================================
        # PHASE C4: AG{rho_x,v_x,u_e,u_x,v_e}; RS(rho_e); AR-max(rho_e)
        # =================================================================
        for i in range(4):
            t5 = sm.tile([128, 5], f32, tag="t5", name="t5")
            for col, src in enumerate([rho_x[i], v_x[i], u_e[i], u_x[i],
                                       v_e[i]]):
                nc.vector.tensor_copy(out=t5[:, col:col + 1], in_=src[:])
            nc.sync.dma_start(agv_in[i * 128:(i + 1) * 128, :], t5[:])
        nc.gpsimd.collective_compute("AllGather", OP.bypass, replica_groups=RG,
                                     ins=[agv_in.ap()], outs=[agv_out.ap()])

        es_re = ExitStack()
        ppre = es_re.enter_context(tc.tile_pool(name="ppre", bufs=2, space="PSUM"))
        rho_b = [sm.tile([128, 1], bf16, tag=f"rhob{i}", name=f"rhob{i}") for i in range(4)]
        for i in range(4):
            nc.vector.tensor_copy(out=rho_b[i][:], in_=rho_x[i][:])
        for m in range(32):
            ps = ppre.tile([128, 1], f32, tag="psre", name="psre")
            for i in range(4):
                nc.tensor.matmul(ps[:], hf[i][:, m * 128:(m + 1) * 128],
                                 rho_b[i][:], start=(i == 0), stop=(i == 3))
            rev = sm.tile([128, 1], f32, tag="rev", name="rev")
            nc.scalar.copy(out=rev[:], in_=ps[:])
            nc.sync.dma_start(re_in[m * 128:(m + 1) * 128, :], rev[:])
        es_re.close()
        nc.gpsimd.collective_compute("ReduceScatter", OP.add, replica_groups=RG,
                                     ins=[re_in.ap()], outs=[re_rs.ap()])
        re_row = rp.tile([1, SH], f32, tag="rowsm", name="rerow")
        nc.sync.dma_start(re_row[:], re_rs.ap())
        mre_l = sm.tile([1, 1], f32, tag="mrel", name="mrel")
        nc.vector.reduce_max(mre_l[:], re_row[:], axis=AX.X)
        mx_t = sm.tile([1, 8], f32, tag="mxt", name="mxt")
        nc.vector.tensor_scalar(out=mx_t[:], in0=ones8[0:1, :], scalar1=mre_l[:],
                                scalar2=None, op0=OP.mult)
        nc.sync.dma_start(mx_in[:, :], mx_t[:])
        nc.gpsimd.collective_compute("AllReduce", OP.max, replica_groups=RG,
                                     ins=[mx_in.ap()], outs=[mx_out.ap()])

        # =================================================================
        # PHASE C5: attention scales; exp_x; denom_x(RS); numer X(RS)
        # =================================================================
        gmax = {}
        for col, nm in [(0, "mrx"), (1, "mvx"), (2, "mue"), (3, "mux"),
                        (4, "mve")]:
            row = rp.tile([1, N], f32, tag="row4k", name="maxrow")
            nc.sync.dma_start(row[:], agv_out.ap()[:, col:col + 1])
            mt = sm.tile([1, 1], f32, tag=f"g{nm}", name=f"g{nm}")
            nc.vector.reduce_max(mt[:], row[:], axis=AX.X)
            gmax[nm] = mt
        mre = sm.tile([1, 1], f32, tag="gmre", name="gmre")
        mrow = sm.tile([1, 8], f32, tag="mrow8", name="mrow8")
        nc.sync.dma_start(mrow[:], mx_out[:, :])
        nc.vector.tensor_copy(out=mre[:], in_=mrow[:, 0:1])

        def lrelu_scalar(nm, a, b):
            s = sm.tile([1, 1], f32, tag=f"ls{nm}", name=f"ls{nm}")
            nc.vector.tensor_tensor(out=s[:], in0=a[:], in1=b[:], op=OP.add)
            s2 = sm.tile([1, 1], f32, tag=f"ls2{nm}", name=f"ls2{nm}")
            nc.vector.tensor_scalar(out=s2[:], in0=s[:], scalar1=SLOPE,
                                    scalar2=None, op0=OP.mult)
            mo = sm.tile([1, 1], f32, tag=f"lm{nm}", name=f"lm{nm}")
            nc.vector.tensor_tensor(out=mo[:], in0=s[:], in1=s2[:], op=OP.max)
            return mo

        def att_scale(nm, maxa, maxr):
            rr = sm.tile([1, 1], f32, tag=f"rr{nm}", name=f"rr{nm}")
            nc.vector.reciprocal(out=rr[:], in_=maxr[:])
            sc = sm.tile([1, 1], f32, tag=f"sc{nm}", name=f"sc{nm}")
            nc.vector.tensor_tensor(out=sc[:], in0=maxa[:], in1=rr[:],
                                    op=OP.mult)
            scb = sm.tile([128, 1], f32, tag=f"scb{nm}", name=f"scb{nm}")
            nc.gpsimd.partition_broadcast(scb[:], sc[:])
            return scb

        max_ax = lrelu_scalar("x", gmax["mux"], gmax["mvx"])
        max_ae = lrelu_scalar("e", gmax["mue"], gmax["mve"])
        scxb = att_scale("x", max_ax, gmax["mrx"])
        sceb = att_scale("e", max_ae, mre)

        def exp_biases(nm, uvec, rvec, scb):
            b1, b2 = [], []
            for i in range(4):
                rt = sm.tile([128, 1], f32, tag=f"rt{nm}{i}", name=f"rt{nm}{i}")
                nc.vector.tensor_scalar(out=rt[:], in0=rvec[i][:],
                                        scalar1=scb[:], scalar2=None,
                                        op0=OP.mult)
                t1 = sm.tile([128, 1], f32, tag=f"b1{nm}{i}", name=f"b1{nm}{i}")
                nc.vector.tensor_tensor(out=t1[:], in0=uvec[i][:], in1=rt[:],
                                        op=OP.add)
                t2 = sm.tile([128, 1], f32, tag=f"b2{nm}{i}", name=f"b2{nm}{i}")
                nc.vector.scalar_tensor_tensor(out=t2[:], in0=uvec[i][:],
                                               scalar=SLOPE, in1=rt[:],
                                               op0=OP.mult, op1=OP.add)
                b1.append(t1)
                b2.append(t2)
            return b1, b2

        rho_e_sl = [sm.tile([128, 1], f32, tag=f"res{i}", name=f"res{i}") for i in range(4)]
        for i in range(4):
            nc.sync.dma_start(rho_e_sl[i][:], re_rs[i * 128:(i + 1) * 128, :])
        b1x, b2x = exp_biases("x", u_x, rho_x, scxb)
        b1e, b2e = exp_biases("e", u_e, rho_e_sl, sceb)

        vb = cpl.tile([128, N], f32, tag="vbcast", name="vbcast")

        def exp_phase(expt, b1, b2):
            for i in range(4):
                for jt in range(NJT):
                    sl_ = (slice(None), slice(jt * JT, (jt + 1) * JT))
                    e1 = tp.tile([128, JT], f32, tag="t512", name="e1t", bufs=4)
                    nc.scalar.activation(out=e1[:], in_=vb[sl_], func=AF.Exp,
                                         bias=b1[i][:], scale=1.0)
                    e2 = tp.tile([128, JT], f32, tag="t512", name="e2t", bufs=4)
                    nc.scalar.activation(out=e2[:], in_=vb[sl_], func=AF.Exp,
                                         bias=b2[i][:], scale=SLOPE)
                    nc.vector.tensor_tensor(out=expt[i][sl_], in0=e1[:],
                                            in1=e2[:], op=OP.max)

        def denom_phase(expt, dn_dram, pp_dn):
            for nt in range(NJT):
                ps = pp_dn.tile([1, JT], f32, tag="psdn", name="psdn")
                for i in range(4):
                    me = tp.tile([128, JT], f32, tag="t512", name="metmp", bufs=4)
                    nc.vector.tensor_tensor(
                        out=me[:], in0=expt[i][:, nt * JT:(nt + 1) * JT],
                        in1=hf[i][:, nt * JT:(nt + 1) * JT], op=OP.mult)
                    nc.tensor.matmul(ps[:], ones_col[:], me[:],
                                     start=(i == 0), stop=(i == 3))
                dr = rp.tile([1, JT], f32, tag="rowsm", name="dnrow")
                nc.scalar.copy(out=dr[:], in_=ps[:])
                nc.sync.dma_start(dn_dram[nt * JT:(nt + 1) * JT, :], dr[:])

        def numer_phase(expt, rhs_tiles, out_dram, pp_nm):
            for m in range(32):
                ps = pp_nm.tile([128, D], f32, tag="psnum", name="psnum")
                for i in range(4):
                    nc.tensor.matmul(ps[:], expt[i][:, m * 128:(m + 1) * 128],
                                     rhs_tiles[i][:], start=(i == 0),
                                     stop=(i == 3))
                ev = tp.tile([128, D], f32, tag="t256", name="numev", bufs=6)
                nc.scalar.copy(out=ev[:], in_=ps[:])
                nc.sync.dma_start(out_dram[m * 128:(m + 1) * 128, :], ev[:])

        vrow = rp.tile([1, N], f32, tag="row4k", name="vrow")
        nc.sync.dma_start(vrow[:], agv_out.ap()[:, 1:2])
        nc.gpsimd.partition_broadcast(vb[:], vrow[:])

        es_ex = ExitStack()
        exp_pool = es_ex.enter_context(tc.tile_pool(name="expx", bufs=4))
        ppX = es_ex.enter_context(tc.tile_pool(name="ppX", bufs=2, space="PSUM"))
        exp_x = [exp_pool.tile([128, N], f32, tag="expx", name="expx") for _ in range(4)]
        exp_phase(exp_x, b1x, b2x)
        denom_phase(exp_x, dnx_in, ppX)
        nc.gpsimd.collective_compute("ReduceScatter", OP.add, replica_groups=RG,
                                     ins=[dnx_in.ap()], outs=[dnx_rs.ap()])
        numer_phase(exp_x, xlw, nx_in, ppX)
        nc.gpsimd.collective_compute("ReduceScatter", OP.add, replica_groups=RG,
                                     ins=[nx_in.ap()], outs=[nx_rs.ap()])
        es_ex.close()

        # =================================================================
        # PHASE C6: E_tilde slab = elu(nx_rs/dnx_rs); exp_e; denom_e; numer E
        # =================================================================
        def elu_tile(out_ap, in_ap, rscale):
            r0 = tp.tile([128, D], f32, tag="t256", name="elur", bufs=6)
            nc.scalar.activation(out=r0[:], in_=in_ap, func=AF.Relu,
                                 scale=rscale)
            m0 = tp.tile([128, D], f32, tag="t256", name="elum", bufs=6)
            nc.vector.tensor_scalar(out=m0[:], in0=in_ap, scalar1=rscale,
                                    scalar2=0.0, op0=OP.mult, op1=OP.min)
            e0 = tp.tile([128, D], f32, tag="t256", name="elue", bufs=6)
            nc.scalar.activation(out=e0[:], in_=m0[:], func=AF.Exp)
            nc.vector.scalar_tensor_tensor(out=out_ap, in0=r0[:], scalar=-1.0,
                                           in1=e0[:], op0=OP.add, op1=OP.add)

        etil = [cpl.tile([128, D], f32, tag=f"et{i}", name=f"et{i}") for i in range(4)]
        for i in range(4):
            nxs = tp.tile([128, D], f32, tag="t256", name="nxs", bufs=6)
            nc.sync.dma_start(nxs[:], nx_rs[i * 128:(i + 1) * 128, :])
            rdx = sm.tile([128, 1], f32, tag="rdx", name="rdx")
            nc.sync.dma_start(rdx[:], dnx_rs[i * 128:(i + 1) * 128, :])
            rdxr = sm.tile([128, 1], f32, tag="rdxr", name="rdxr")
            nc.vector.reciprocal(out=rdxr[:], in_=rdx[:])
            elu_tile(etil[i][:], nxs[:], rdxr[:])

        vrow_e = rp.tile([1, N], f32, tag="row4k", name="vrowe")
        nc.sync.dma_start(vrow_e[:], agv_out.ap()[:, 4:5])
        nc.gpsimd.partition_broadcast(vb[:], vrow_e[:])

        es_ee = ExitStack()
        expe_pool = es_ee.enter_context(tc.tile_pool(name="expe", bufs=4))
        ppE = es_ee.enter_context(tc.tile_pool(name="ppE", bufs=2, space="PSUM"))
        exp_e = [expe_pool.tile([128, N], f32, tag="expe", name="expe") for _ in range(4)]
        exp_phase(exp_e, b1e, b2e)
        denom_phase(exp_e, dne_in, ppE)
        nc.gpsimd.collective_compute("AllReduce", OP.add, replica_groups=RG,
                                     ins=[dne_in.ap()], outs=[dne_ar.ap()])
        numer_phase(exp_e, etil, ne_in, ppE)
        nc.gpsimd.collective_compute("AllReduce", OP.add, replica_groups=RG,
                                     ins=[ne_in.ap()], outs=[ne_ar.ap()])
        es_ee.close()
        es_c.close()
        es_hf.close()

        # =================================================================
        # PHASE C7: X_tilde = elu(ne_ar / dne_ar) -> out
        # =================================================================
        for m in range(32):
            nes = tp.tile([128, D], f32, tag="t256", name="nxs", bufs=6)
            nc.sync.dma_start(nes[:], ne_ar[m * 128:(m + 1) * 128, :])
            rde = sm.tile([128, 1], f32, tag="rde", name="rde")
            nc.sync.dma_start(rde[:], dne_ar[m * 128:(m + 1) * 128, :])
            rder = sm.tile([128, 1], f32, tag="rder", name="rder")
            nc.vector.reciprocal(out=rder[:], in_=rde[:])
            ot = tp.tile([128, D], f32, tag="t256", name="otile", bufs=6)
            elu_tile(ot[:], nes[:], rder[:])
            nc.sync.dma_start(out_t[m * 128:(m + 1) * 128, :], ot[:])

    nc.compile()
    return nc


_CACHE = {}


def _get_prog():
    if "nc" not in _CACHE:
        _CACHE["nc"] = _build()
    return _CACHE["nc"]


def make_in_maps(X, theta, W, alpha_x, alpha_e):
    X = np.ascontiguousarray(np.asarray(X, np.float32))
    theta = np.ascontiguousarray(np.asarray(theta, np.float32))
    W = np.ascontiguousarray(np.asarray(W, np.float32))
    alpha_x = np.asarray(alpha_x, np.float32).reshape(-1)
    alpha_e = np.asarray(alpha_e, np.float32).reshape(-1)
    XT = np.ascontiguousarray(X.T)                       # [784, 4096]
    xt7 = np.ascontiguousarray(XT.reshape(NKF, KCH, N))
    th7 = np.ascontiguousarray(theta.reshape(NKF, KCH, D))
    w2 = np.ascontiguousarray(W.reshape(2, 128, D))
    al = np.ascontiguousarray(np.stack([alpha_x, alpha_e]))  # [2, 512]
    in_maps = []
    for c in range(NCORE):
        xtc = np.ascontiguousarray(xt7[:, :, c * SH:(c + 1) * SH])
        in_maps.append({"xt": xt7, "xtc": xtc, "theta": th7, "w": w2,
                        "alpha": al})
    return in_maps


def kernel(X, theta, W, alpha_x, alpha_e):
    nc = _get_prog()
    in_maps = make_in_maps(X, theta, W, alpha_x, alpha_e)
    res = run_bass_kernel_spmd(nc, in_maps, core_ids=list(range(NCORE)))
    return np.asarray(res.results[0]["out"], np.float32)



# revision 31
# speedup vs baseline: 1.8844x; 1.8844x over previous
"""DA-HGNN forward kernel, row-sharded SPMD across 8 Trainium2 NeuronCores.

Self-contained: takes full inputs, shards host-side, runs one Bass/Tile
program on cores 0-7 with collectives, returns the full [4096, 256] output
(host concatenates the per-core 512-row shards).

Key optimizations over the baseline:
- scores matmul via bf16 hi/lo split (3 passes at 1 cycle/row, ~2^-16
  accuracy) instead of fp32's 4 cycles/row; Y / S1 / G / XlW / denom /
  numer matmuls in bf16
- H^T stored as fp8(e4m3); adjacency and E matmuls use fp8 DoubleRow
  (0.5 cycles/row); H^T AllGather payload halves and is split in two so it
  overlaps phase-A compute
- -|x|^2/2 bias row and W@alpha vectors precomputed on host (kills one
  collective)
- rho_e via 32 wide [1,512] matmuls + one AllReduce + local max (replaces
  ReduceScatter + AllReduce-max chain); per-core shard extracted with a
  one-hot select matmul
- numerator+denominator packed into one [8,514,256] ReduceScatter per
  attention; exp_e/denom_e overlap the x-side ReduceScatter
- each core emits only its 512-row output shard (no final AllGather)
"""
import numpy as np

from contextlib import ExitStack

from concourse import bass, mybir, bacc, tile
from concourse.bass_utils import run_bass_kernel_spmd

f32 = mybir.dt.float32
f32r = mybir.dt.float32r
bf16 = mybir.dt.bfloat16
fp8 = mybir.dt.float8e4
u32 = mybir.dt.uint32
AF = mybir.ActivationFunctionType
OP = mybir.AluOpType
AX = mybir.AxisListType
PM = mybir.MatmulPerfMode

N = 4096          # nodes == hyperedges
F = 784           # input features
D = 256           # hidden dim
NCORE = 8
SH = N // NCORE   # 512 rows per core
KCH = 112         # 7 chunks of 112 over F
NKF = 7
TOPK = 11
SIGMA = 0.3
SLOPE = 0.2
DV = float(np.float32(1.0) / np.sqrt(np.float32(TOPK)))
NEG_BIG = -3.0e38

JW = 512          # phase-A slab width
NJW = N // JW     # 8
JT = 512          # j-tile width for exp phases
NJT = N // JT     # 8



def _build():
    nc = bacc.Bacc("TRN2", target_bir_lowering=False, debug=False,
                   num_devices=NCORE)

    # ---- I/O -------------------------------------------------------------
    xth_in = nc.dram_tensor("xth", [NKF, KCH, N], bf16, kind="ExternalInput")
    xtl_in = nc.dram_tensor("xtl", [NKF, KCH, N], bf16, kind="ExternalInput")
    nsqh_in = nc.dram_tensor("nsqh", [1, N], bf16, kind="ExternalInput")
    nsql_in = nc.dram_tensor("nsql", [1, N], bf16, kind="ExternalInput")
    xtch_in = nc.dram_tensor("xtch", [NKF, KCH, SH], bf16, kind="ExternalInput")
    xtcl_in = nc.dram_tensor("xtcl", [NKF, KCH, SH], bf16, kind="ExternalInput")
    th_in = nc.dram_tensor("theta", [NKF, KCH, D], bf16, kind="ExternalInput")
    w_in = nc.dram_tensor("w", [2, 128, D], bf16, kind="ExternalInput")
    wv_in = nc.dram_tensor("wv", [2, D], f32, kind="ExternalInput")
    al_in = nc.dram_tensor("alpha", [2, 2 * D], f32, kind="ExternalInput")
    myhot_in = nc.dram_tensor("myhot", [NCORE, 1], f32, kind="ExternalInput")
    out_t = nc.dram_tensor("out", [SH, D], f32, kind="ExternalOutput")

    # ---- internal DRAM (collective bounces) ------------------------------
    hbt_a = nc.dram_tensor("hbt_a", [N, 256], fp8)
    hbt_b = nc.dram_tensor("hbt_b", [N, 256], fp8)
    ht_a = nc.dram_tensor("ht_a", [N * NCORE, 256], fp8, addr_space="Shared")
    ht_b = nc.dram_tensor("ht_b", [N * NCORE, 256], fp8, addr_space="Shared")
    s1_io = nc.dram_tensor("s1_io", [N, D + 1], bf16)
    s1_rs = nc.dram_tensor("s1_rs", [SH, D + 1], bf16)
    s1_full = nc.dram_tensor("s1_full", [N, D + 1], bf16, addr_space="Shared")
    agv_in = nc.dram_tensor("agv_in", [5, SH], f32)
    agv_out = nc.dram_tensor("agv_out", [5 * NCORE, SH], f32,
                             addr_space="Shared")
    re_io = nc.dram_tensor("re_io", [1, N], f32)
    re_ar = nc.dram_tensor("re_ar", [1, N], f32, addr_space="Shared")
    nxd_in = nc.dram_tensor("nxd_in", [NCORE, SH + 2, D], bf16)
    nxd_rs = nc.dram_tensor("nxd_rs", [SH + 2, D], bf16)
    ned_in = nc.dram_tensor("ned_in", [NCORE, SH + 2, D], bf16)
    ned_rs = nc.dram_tensor("ned_rs", [SH + 2, D], bf16)

    RG = [list(range(NCORE))]

    with tile.TileContext(nc) as tc, ExitStack() as top:
        cp = top.enter_context(tc.tile_pool(name="const", bufs=1))
        sm = top.enter_context(tc.tile_pool(name="smalls", bufs=2))
        tp = top.enter_context(tc.tile_pool(name="tmps", bufs=3))
        rp = top.enter_context(tc.tile_pool(name="rows", bufs=1))

        def rsqrt_(out_ap, in_ap, scale, shape):
            t_ = sm.tile(shape, f32, tag="rsqt", name="rsqt")
            nc.scalar.activation(out=t_[:], in_=in_ap, func=AF.Sqrt, scale=scale)
            nc.vector.reciprocal(out=out_ap, in_=t_[:])

        # constants
        ident = cp.tile([128, 128], f32, tag="ident", name="ident")
        ident_b = cp.tile([128, 128], bf16, tag="identb", name="identb")
        ones_b = cp.tile([128, 1], bf16, tag="onesb", name="onesb")
        nc.vector.memset(ones_b[:], 1.0)
        ones8 = cp.tile([128, 8], f32, tag="ones8", name="ones8")
        nc.vector.memset(ones8[:], 1.0)

        w_sb = [cp.tile([128, D], bf16, tag=f"w{k}", name=f"w{k}") for k in range(2)]
        for k in range(2):
            nc.sync.dma_start(w_sb[k][:], w_in[k, :, :])

        # long-lived big tensors
        es_hf = ExitStack()
        hfp = es_hf.enter_context(tc.tile_pool(name="hfinal", bufs=4))
        hf = [hfp.tile([128, N], bf16, tag="hf", name="hf") for _ in range(4)]
        es_xtc = ExitStack()
        xp = es_xtc.enter_context(tc.tile_pool(name="xtc", bufs=1))
        xtch = [xp.tile([KCH + (1 if k == 0 else 0), SH], bf16, tag=f"xtch{k}", name=f"xtch{k}")
                for k in range(NKF)]
        xtcl = [xp.tile([KCH + (1 if k == 0 else 0), SH], bf16, tag=f"xtcl{k}", name=f"xtcl{k}")
                for k in range(NKF)]
        nc.vector.memset(xtch[0][:, :], 1.0)  # hi row 112 stays ones
        nc.vector.memset(xtcl[0][:, :], 0.0)  # lo row 112 stays zero
        for k in range(NKF):
            nc.sync.dma_start(xtch[k][0:KCH, :], xtch_in[k, :, :])
            nc.sync.dma_start(xtcl[k][0:KCH, :], xtcl_in[k, :, :])

        # phase-B pools created before phase-A pools (LIFO close order)
        es_b = ExitStack()
        bp = es_b.enter_context(tc.tile_pool(name="bphase", bufs=1))
        bp2 = es_b.enter_context(tc.tile_pool(name="bphase2", bufs=3))
        ppB = es_b.enter_context(tc.tile_pool(name="ppB", bufs=2, space="PSUM"))
        thsb = [bp.tile([KCH, D], bf16, tag=f"th{k}", name=f"th{k}") for k in range(NKF)]
        for k in range(NKF):
            nc.sync.dma_start(thsb[k][:], th_in[k, :, :])

        # =================================================================
        # PHASE A: scores = Xc @ X.T - sq/2 ; top-k -> H ; H^T(fp8) ; 2x AG
        # =================================================================
        es_a = ExitStack()
        ap_ = es_a.enter_context(tc.tile_pool(name="aphase", bufs=1))
        ap2 = es_a.enter_context(tc.tile_pool(name="aphase2", bufs=2))
        ppA = es_a.enter_context(tc.tile_pool(name="ppA", bufs=3, space="PSUM"))
        ppT = es_a.enter_context(tc.tile_pool(name="ppTa", bufs=2, space="PSUM"))

        io128 = ap2.tile([128, 128], f32, tag="io128", name="io128", bufs=1)
        nc.gpsimd.iota(io128[:], pattern=[[1, 128]], base=0, channel_multiplier=-1,
                       allow_small_or_imprecise_dtypes=True)
        nc.vector.tensor_scalar(out=ident[:], in0=io128[:], scalar1=0.0,
                                scalar2=None, op0=OP.is_equal)
        nc.vector.tensor_copy(out=ident_b[:], in_=ident[:])

        iota5 = ap_.tile([128, 512], f32, tag="iota5", name="iota5")
        nc.gpsimd.iota(iota5[:], pattern=[[1, 512]], base=0, channel_multiplier=0,
                       allow_small_or_imprecise_dtypes=True)

        scores = [ap_.tile([128, N], f32, tag=f"sc{i}", name=f"sc{i}") for i in range(3)]

        def score_block(half):
            for j in range(NJW):
                jsl = slice(j * JW, (j + 1) * JW)
                slh = [ap2.tile([KCH + (1 if k == 0 else 0), JW], bf16,
                                tag=f"slh{k}", name=f"slh{k}") for k in range(NKF)]
                sll = [ap2.tile([KCH + (1 if k == 0 else 0), JW], bf16,
                                tag=f"sll{k}", name=f"sll{k}") for k in range(NKF)]
                for k in range(NKF):
                    nc.sync.dma_start(slh[k][0:KCH, :], xth_in[k, :, jsl])
                    nc.sync.dma_start(sll[k][0:KCH, :], xtl_in[k, :, jsl])
                nc.sync.dma_start(slh[0][KCH:KCH + 1, :], nsqh_in[0:1, jsl])
                nc.sync.dma_start(sll[0][KCH:KCH + 1, :], nsql_in[0:1, jsl])
                for ii in range(2):
                    i = 2 * half + ii
                    isl = slice(i * 128, (i + 1) * 128)
                    ps = ppA.tile([128, JW], f32, tag="psA", name="psA")
                    nmm = 3 * NKF
                    mi = 0
                    for lhs_set, rhs_set in ((xtch, slh), (xtch, sll),
                                             (xtcl, slh)):
                        for k in range(NKF):
                            kk = KCH + (1 if k == 0 else 0)
                            nc.tensor.matmul(ps[:], lhs_set[k][0:kk, isl],
                                             rhs_set[k][0:kk, :],
                                             start=(mi == 0),
                                             stop=(mi == nmm - 1))
                            mi += 1
                    nc.scalar.copy(out=scores[i % 3][:, jsl], in_=ps[:])

        def topk_block(i):
            # top-k threshold + exact tie-break -> H rows (bf16 0/1)
            sc = scores[i % 3]
            m1 = sm.tile([128, 8], f32, tag="m1", name="m1")
            m2 = sm.tile([128, 8], f32, tag="m2", name="m2")
            tmpf = scores[(i + 2) % 3]  # idle buffer this round
            nc.vector.max(m1[:], sc[:])
            nc.vector.match_replace(tmpf[:], m1[:], sc[:], NEG_BIG)
            nc.vector.max(m2[:], tmpf[:])
            tq = m2[:, 2:3]  # 11th largest
            hA = ap2.tile([128, N], bf16, tag="hwork", name="hwork")
            nc.vector.tensor_scalar(out=hA[:], in0=sc[:], scalar1=tq,
                                    scalar2=None, op0=OP.is_gt)
            cst = sm.tile([128, 1], f32, tag="cst", name="cst")
            nc.vector.reduce_sum(cst[:], hA[:], axis=AX.X)
            need = sm.tile([128, 1], f32, tag="need", name="need")
            nc.vector.tensor_scalar(out=need[:], in0=cst[:], scalar1=-1.0,
                                    scalar2=float(TOPK), op0=OP.mult,
                                    op1=OP.add)
            t8 = sm.tile([128, 8], f32, tag="t8", name="t8")
            nc.vector.tensor_scalar(out=t8[:], in0=ones8[:], scalar1=tq,
                                    scalar2=None, op0=OP.mult)
            idx8 = sm.tile([128, 8], u32, tag="idx8", name="idx8")
            nc.vector.max_index(idx8[:], t8[:], sc[:])
            idxf = sm.tile([128, 8], f32, tag="idxf", name="idxf")
            nc.vector.tensor_copy(out=idxf[:], in_=idx8[:])
            gate1 = sm.tile([128, 1], f32, tag="gate1", name="gate1")
            nc.vector.tensor_scalar(out=gate1[:], in0=need[:], scalar1=1.5,
                                    scalar2=None, op0=OP.is_gt)
            gm1 = sm.tile([128, 1], f32, tag="gm1", name="gm1")
            nc.vector.tensor_scalar(out=gm1[:], in0=gate1[:], scalar1=-1.0,
                                    scalar2=None, op0=OP.add)
            idx1g = sm.tile([128, 1], f32, tag="idx1g", name="idx1g")
            nc.vector.scalar_tensor_tensor(out=idx1g[:], in0=idxf[:, 1:2],
                                           scalar=gate1[:], in1=gm1[:],
                                           op0=OP.mult, op1=OP.add)
            hB = ap2.tile([128, N], bf16, tag="hwork", name="hwork")
            for tb in range(8):
                tsl = slice(tb * 512, (tb + 1) * 512)
                i0a = sm.tile([128, 1], f32, tag="i0a", name="i0a")
                nc.vector.tensor_scalar(out=i0a[:], in0=idxf[:, 0:1],
                                        scalar1=float(-tb * 512),
                                        scalar2=None, op0=OP.add)
                i1a = sm.tile([128, 1], f32, tag="i1a", name="i1a")
                nc.vector.tensor_scalar(out=i1a[:], in0=idx1g[:],
                                        scalar1=float(-tb * 512),
                                        scalar2=None, op0=OP.add)
                nc.vector.scalar_tensor_tensor(out=hB[:, tsl], in0=iota5[:],
                                               scalar=i0a[:], in1=hA[:, tsl],
                                               op0=OP.is_equal, op1=OP.add)
                nc.vector.scalar_tensor_tensor(out=hf[i][:, tsl],
                                               in0=iota5[:], scalar=i1a[:],
                                               in1=hB[:, tsl],
                                               op0=OP.is_equal, op1=OP.add)

        def transp_block(i):
            # transpose H rows -> H^T fp8 columns, DMA out
            dst = hbt_a if i < 2 else hbt_b
            c0 = (i % 2) * 128
            for ec in range(32):
                pt = ppT.tile([128, 128], bf16, tag="ptp", name="ptp")
                nc.tensor.transpose(pt[:], hf[i][:, ec * 128:(ec + 1) * 128],
                                    ident_b[:])
                hev = ap2.tile([128, 128], fp8, tag="hbtev", name="hbtev",
                               bufs=3)
                nc.scalar.copy(out=hev[:], in_=pt[:])
                nc.sync.dma_start(
                    dst[ec * 128:(ec + 1) * 128, c0:c0 + 128], hev[:])

        score_block(0)
        topk_block(0)
        topk_block(1)
        score_block(1)          # tensor overlaps topk(0/1) vector work
        topk_block(2)
        topk_block(3)
        transp_block(0)
        transp_block(1)
        nc.gpsimd.collective_compute("AllGather", OP.bypass, replica_groups=RG,
                                     ins=[hbt_a.ap()], outs=[ht_a.ap()])

        # Y = Xc @ theta: independent tensor work while topk(2/3) finish
        yplus = [bp.tile([128, D + 1], bf16, tag=f"yp{i}", name=f"yp{i}") for i in range(4)]
        for i in range(4):
            ps = ppB.tile([128, D + 1], f32, tag="psB", name="psY")
            for k in range(NKF):
                nc.tensor.matmul(ps[:, 0:D],
                                 xtch[k][0:KCH, :][:, i * 128:(i + 1) * 128],
                                 thsb[k][:], start=(k == 0),
                                 stop=(k == NKF - 1))
            nc.scalar.copy(out=yplus[i][:, 0:D], in_=ps[:, 0:D])
            nc.vector.memset(yplus[i][:, D:D + 1], 1.0)

        transp_block(2)
        transp_block(3)
        nc.gpsimd.collective_compute("AllGather", OP.bypass, replica_groups=RG,
                                     ins=[hbt_b.ap()], outs=[ht_b.ap()])
        es_a.close()

        # =================================================================
        # PHASE B: S1 = H^T @ [Y|1] -> RS + AG
        # =================================================================
        for m in range(32):
            ps = ppB.tile([128, D + 1], f32, tag="psB", name="psS1")
            for i in range(4):
                nc.tensor.matmul(ps[:], hf[i][:, m * 128:(m + 1) * 128],
                                 yplus[i][:, :], start=(i == 0), stop=(i == 3))
            s1t = bp2.tile([128, D + 1], bf16, tag="s1ev", name="s1ev")
            nc.scalar.copy(out=s1t[:], in_=ps[:])
            nc.sync.dma_start(s1_io[m * 128:(m + 1) * 128, :], s1t[:])
        nc.gpsimd.collective_compute("ReduceScatter", OP.add, replica_groups=RG,
                                     ins=[s1_io.ap()], outs=[s1_rs.ap()])
        nc.gpsimd.collective_compute("AllGather", OP.bypass, replica_groups=RG,
                                     ins=[s1_rs.ap()], outs=[s1_full.ap()])
        es_b.close()
        es_xtc.close()

        # =================================================================
        # PHASE C1: my Xl slab; XlcT; XlW; u_x, v_e; sigma*n, 1/n, diag
        # =================================================================
        es_c = ExitStack()
        cpl = es_c.enter_context(tc.tile_pool(name="cphase", bufs=1))
        es_hbt = ExitStack()
        hbtp = es_hbt.enter_context(tc.tile_pool(name="hbt", bufs=1))
        es_xht = ExitStack()
        xhp = es_xht.enter_context(tc.tile_pool(name="xht", bufs=1))
        es_ppc = ExitStack()
        ppC = es_ppc.enter_context(tc.tile_pool(name="ppC", bufs=2, space="PSUM"))
        es_tp = ExitStack()
        ppTf = es_tp.enter_context(tc.tile_pool(name="ppTf", bufs=2, space="PSUM"))

        al_x = cpl.tile([1, 2 * D], f32, tag="alx", name="alx")
        nc.sync.dma_start(al_x[:], al_in[0:1, :])
        al_e = cpl.tile([1, 2 * D], f32, tag="ale", name="ale")
        nc.sync.dma_start(al_e[:], al_in[1:2, :])
        xlc = [cpl.tile([128, D], f32, tag=f"xlc{i}", name=f"xlc{i}") for i in range(4)]
        sgn = [sm.tile([128, 1], f32, tag=f"sgn{i}", name=f"sgn{i}") for i in range(4)]
        rcn = [sm.tile([128, 1], f32, tag=f"rcn{i}", name=f"rcn{i}") for i in range(4)]
        diag = [sm.tile([128, 1], f32, tag=f"diag{i}", name=f"diag{i}") for i in range(4)]
        for i in range(4):
            sl = tp.tile([128, D + 1], bf16, tag="slabs1", name="slabs1")
            nc.sync.dma_start(sl[:], s1_rs[i * 128:(i + 1) * 128, :])
            slf = tp.tile([128, D + 1], f32, tag="slabs1f", name="slabs1f")
            nc.vector.tensor_copy(out=slf[:], in_=sl[:])
            dde = sm.tile([128, 1], f32, tag="dde", name="dde")
            rsqrt_(dde[:], slf[:, D:D + 1], float(TOPK), [128, 1])
            nc.vector.tensor_scalar(out=xlc[i][:], in0=slf[:, 0:D],
                                    scalar1=dde[:], scalar2=None, op0=OP.mult)
            nsq = sm.tile([128, 1], f32, tag="nsq", name="nsq")
            tr = tp.tile([128, D], f32, tag="t256", name="trsq", bufs=6)
            nc.scalar.activation(out=tr[:], in_=xlc[i][:], func=AF.Square,
                                 accum_out=nsq[:])
            nc.scalar.activation(out=sgn[i][:], in_=nsq[:], func=AF.Sqrt,
                                 scale=float(SIGMA) * float(SIGMA))
            rsqrt_(rcn[i][:], nsq[:], 1.0, [128, 1])
            xhc = tp.tile([128, D], f32, tag="t256", name="xhc", bufs=6)
            nc.vector.tensor_scalar(out=xhc[:], in0=xlc[i][:], scalar1=rcn[i][:],
                                    scalar2=None, op0=OP.mult)
            tr2 = tp.tile([128, D], f32, tag="t256", name="trsq", bufs=6)
            nc.vector.scalar_tensor_tensor(out=tr2[:], in0=xlc[i][:], scalar=1.0,
                                           in1=xhc[:], op0=OP.mult, op1=OP.mult,
                                           accum_out=diag[i][:])

        xlct = [cpl.tile([128, SH], bf16, tag=f"xlct{d}", name=f"xlct{d}") for d in range(2)]
        for i in range(4):
            xlcb = tp.tile([128, D], bf16, tag="t256b", name="xlcb", bufs=6)
            nc.vector.tensor_copy(out=xlcb[:], in_=xlc[i][:])
            for d in range(2):
                pt = ppTf.tile([128, 128], bf16, tag="ptpf", name="ptpf")
                nc.tensor.transpose(pt[:], xlcb[:, d * 128:(d + 1) * 128],
                                    ident_b[:])
                nc.scalar.copy(out=xlct[d][:, i * 128:(i + 1) * 128], in_=pt[:])

        xlw = [cpl.tile([128, D], bf16, tag=f"xlw{i}", name=f"xlw{i}") for i in range(4)]
        al_xb = cpl.tile([1, 2 * D], bf16, tag="alxb", name="alxb")
        nc.vector.tensor_copy(out=al_xb[:], in_=al_x[:])
        al_eb = cpl.tile([1, 2 * D], bf16, tag="aleb", name="aleb")
        nc.vector.tensor_copy(out=al_eb[:], in_=al_e[:])
        ax1b = cpl.tile([128, D], bf16, tag="ax1b", name="ax1b")
        nc.gpsimd.partition_broadcast(ax1b[:], al_xb[:, 0:D])
        ae2b = cpl.tile([128, D], bf16, tag="ae2b", name="ae2b")
        nc.gpsimd.partition_broadcast(ae2b[:], al_eb[:, D:2 * D])
        u_x = [sm.tile([128, 1], f32, tag=f"ux{i}", name=f"ux{i}") for i in range(4)]
        v_e = [sm.tile([128, 1], f32, tag=f"ve{i}", name=f"ve{i}") for i in range(4)]
        for i in range(4):
            ps = ppC.tile([128, D], f32, tag="psC", name="psXW")
            for k in range(2):
                nc.tensor.matmul(ps[:], xlct[k][:, i * 128:(i + 1) * 128],
                                 w_sb[k][:], start=(k == 0), stop=(k == 1))
            nc.scalar.copy(out=xlw[i][:], in_=ps[:])
            t1 = tp.tile([128, D], bf16, tag="t256b", name="uvtmp", bufs=6)
            nc.vector.scalar_tensor_tensor(out=t1[:], in0=xlw[i][:], scalar=1.0,
                                           in1=ax1b[:], op0=OP.mult, op1=OP.mult,
                                           accum_out=u_x[i][:])
            t2 = tp.tile([128, D], bf16, tag="t256b", name="uvtmp", bufs=6)
            nc.vector.scalar_tensor_tensor(out=t2[:], in0=xlw[i][:], scalar=1.0,
                                           in1=ae2b[:], op0=OP.mult, op1=OP.mult,
                                           accum_out=v_e[i][:])

        # =================================================================
        # PHASE C2: full pass -> X^lT (G rhs), Z(fp8);  E = dv*H_c@Z
        # =================================================================
        hbt = hbtp.tile([128, 32, SH], fp8, tag="hbt", name="hbt")
        nc.sync.dma_start(hbt[:, :, 0:256],
                          hbt_a.ap().rearrange("(ec p) i -> p ec i", p=128))
        nc.sync.dma_start(hbt[:, :, 256:512],
                          hbt_b.ap().rearrange("(ec p) i -> p ec i", p=128))
        xht = [xhp.tile([128, N], bf16, tag=f"xht{d}", name=f"xht{d}") for d in range(2)]
        es_z = ExitStack()
        zp = es_z.enter_context(tc.tile_pool(name="zp", bufs=1))
        z8 = zp.tile([128, 32, D], fp8, tag="z8", name="z8")
        for m in range(32):
            sl = tp.tile([128, D + 1], bf16, tag="slabs1", name="slabs1")
            nc.sync.dma_start(sl[:], s1_full[m * 128:(m + 1) * 128, :])
            slf = tp.tile([128, D + 1], f32, tag="slabs1f", name="slabs1f")
            nc.vector.tensor_copy(out=slf[:], in_=sl[:])
            dde = sm.tile([128, 1], f32, tag="dde", name="dde")
            rsqrt_(dde[:], slf[:, D:D + 1], float(TOPK), [128, 1])
            de1 = sm.tile([128, 1], f32, tag="de1", name="de1")
            rsqrt_(de1[:], slf[:, D:D + 1], 1.0, [128, 1])
            xlm = tp.tile([128, D], f32, tag="t256", name="xlm", bufs=6)
            nc.vector.tensor_scalar(out=xlm[:], in0=slf[:, 0:D], scalar1=dde[:],
                                    scalar2=None, op0=OP.mult)
            nc.scalar.activation(out=z8[:, m, :], in_=xlm[:], func=AF.Copy,
                                 scale=de1[:])
            nsq = sm.tile([128, 1], f32, tag="nsq", name="nsq")
            tr = tp.tile([128, D], f32, tag="t256", name="trsq", bufs=6)
            nc.scalar.activation(out=tr[:], in_=xlm[:], func=AF.Square,
                                 accum_out=nsq[:])
            rc = sm.tile([128, 1], f32, tag="rcm", name="rcm")
            rsqrt_(rc[:], nsq[:], 1.0, [128, 1])
            xhm = tp.tile([128, D], bf16, tag="t256b", name="xhm", bufs=6)
            nc.vector.tensor_scalar(out=xhm[:], in0=xlm[:], scalar1=rc[:],
                                    scalar2=None, op0=OP.mult)
            for d in range(2):
                pt = ppTf.tile([128, 128], bf16, tag="ptpf", name="ptpf")
                nc.tensor.transpose(pt[:], xhm[:, d * 128:(d + 1) * 128],
                                    ident_b[:])
                nc.scalar.copy(out=xht[d][:, m * 128:(m + 1) * 128], in_=pt[:])

        e_c = [cpl.tile([128, D], bf16, tag=f"ec{i}", name=f"ec{i}") for i in range(4)]
        for i in range(4):
            ps = ppC.tile([128, D], f32, tag="psC", name="psE")
            for ec in range(0, 32, 2):
                nc.tensor.matmul(ps[:], hbt[:, ec:ec + 2, i * 128:(i + 1) * 128],
                                 z8[:, ec:ec + 2, :], start=(ec == 0),
                                 stop=(ec == 30), perf_mode=PM.DoubleRow)
            nc.scalar.activation(out=e_c[i][:], in_=ps[:], func=AF.Copy, scale=DV)
        es_z.close()

        # w-vectors (host-precomputed): wx2 = W @ alpha_x[256:], we1 = W @ alpha_e[:256]
        wx2b = cpl.tile([128, D], bf16, tag="wx2b", name="wx2b")
        we1b = cpl.tile([128, D], bf16, tag="we1b", name="we1b")
        wvrow = rp.tile([1, D], f32, tag="rowsm", name="wvrow")
        nc.sync.dma_start(wvrow[:], wv_in[0:1, :])
        wvrow_b = rp.tile([1, D], bf16, tag="rowsmb", name="wvrowb")
        nc.vector.tensor_copy(out=wvrow_b[:], in_=wvrow[:])
        nc.gpsimd.partition_broadcast(wx2b[:], wvrow_b[:])
        wvrow2 = rp.tile([1, D], f32, tag="rowsm2", name="wvrow2")
        nc.sync.dma_start(wvrow2[:], wv_in[1:2, :])
        wvrow2_b = rp.tile([1, D], bf16, tag="rowsm2b", name="wvrow2b")
        nc.vector.tensor_copy(out=wvrow2_b[:], in_=wvrow2[:])
        nc.gpsimd.partition_broadcast(we1b[:], wvrow2_b[:])
        u_e = [sm.tile([128, 1], f32, tag=f"ue{i}", name=f"ue{i}") for i in range(4)]
        v_x = [sm.tile([128, 1], f32, tag=f"vx{i}", name=f"vx{i}") for i in range(4)]
        for i in range(4):
            t1 = tp.tile([128, D], bf16, tag="t256b", name="uvtmp", bufs=6)
            nc.vector.scalar_tensor_tensor(out=t1[:], in0=e_c[i][:], scalar=1.0,
                                           in1=we1b[:], op0=OP.mult, op1=OP.mult,
                                           accum_out=u_e[i][:])
            t2 = tp.tile([128, D], bf16, tag="t256b", name="uvtmp", bufs=6)
            nc.vector.scalar_tensor_tensor(out=t2[:], in0=e_c[i][:], scalar=1.0,
                                           in1=wx2b[:], op0=OP.mult, op1=OP.mult,
                                           accum_out=v_x[i][:])

        # =================================================================
        # PHASE C3: adjacency(fp8 DoubleRow) + masked cosine rho
        # =================================================================
        NJQ = 16
        rho_x = [sm.tile([128, 1], f32, tag=f"rho{i}", name=f"rho{i}") for i in range(4)]
        rho_cols = [cpl.tile([128, NJQ], f32, tag=f"rhoc{i}", name=f"rhoc{i}") for i in range(4)]
        es_slab = ExitStack()
        slp = es_slab.enter_context(tc.tile_pool(name="htslab", bufs=3))
        rtp = es_slab.enter_context(tc.tile_pool(name="rhotmp", bufs=2))
        ppR = es_slab.enter_context(tc.tile_pool(name="ppR", bufs=2, space="PSUM"))
        for jq in range(NJQ):
            r_, ch = jq // 2, jq % 2
            hsrc = ht_a if ch == 0 else ht_b
            hts = []
            for eh in range(2):
                h_ = slp.tile([128, 16, 256], fp8, tag="hts", name="hts")
                nc.sync.dma_start(
                    h_[:], hsrc[r_ * N + eh * 2048:r_ * N + (eh + 1) * 2048, :]
                    .rearrange("(ec p) i -> p ec i", p=128))
                hts.append(h_)
            for i in range(4):
                psA = ppR.tile([128, 256], f32, tag="psadj", name="psadj")
                for eh in range(2):
                    for ec in range(0, 16, 2):
                        nc.tensor.matmul(
                            psA[:],
                            hbt[:, eh * 16 + ec:eh * 16 + ec + 2,
                                i * 128:(i + 1) * 128],
                            hts[eh][:, ec:ec + 2, :],
                            start=(eh == 0 and ec == 0),
                            stop=(eh == 1 and ec == 14),
                            perf_mode=PM.DoubleRow)
                psG = ppR.tile([128, 256], f32, tag="psg", name="psg")
                for k in range(2):
                    nc.tensor.matmul(psG[:], xlct[k][:, i * 128:(i + 1) * 128],
                                     xht[k][:, jq * 256:(jq + 1) * 256],
                                     start=(k == 0), stop=(k == 1))
                g_sb = rtp.tile([128, 256], f32, tag="gsb", name="gsb", bufs=1)
                nc.scalar.copy(out=g_sb[:], in_=psG[:])
                t1 = rtp.tile([128, 256], f32, tag="rt1", name="rt1", bufs=1)
                nc.vector.scalar_tensor_tensor(out=t1[:], in0=psA[:],
                                               scalar=0.5, in1=g_sb[:],
                                               op0=OP.is_gt, op1=OP.mult)
                t2 = rtp.tile([128, 256], f32, tag="rt2", name="rt2", bufs=1)
                nc.vector.scalar_tensor_tensor(out=t2[:], in0=g_sb[:],
                                               scalar=sgn[i][:], in1=t1[:],
                                               op0=OP.is_gt, op1=OP.mult,
                                               accum_out=rho_cols[i][:, jq:jq + 1])
        es_slab.close()
        es_xht.close()
        es_hbt.close()
        es_tp.close()
        es_ppc.close()
        for i in range(4):
            r1 = sm.tile([128, 1], f32, tag="r1", name="r1")
            nc.vector.reduce_sum(r1[:], rho_cols[i][:], axis=AX.X)
            r2 = sm.tile([128, 1], f32, tag="r2", name="r2")
            nc.vector.tensor_tensor(out=r2[:], in0=r1[:], in1=diag[i][:],
                                    op=OP.subtract)
            nc.vector.tensor_tensor(out=rho_x[i][:], in0=r2[:], in1=rcn[i][:],
                                    op=OP.mult)

        # =================================================================
        # PHASE C4: AG{rho_x,v_x,u_e,u_x,v_e} (row layout); AR(rho_e row)
        # =================================================================
        es_c4 = ExitStack()
        r5 = es_c4.enter_context(tc.tile_pool(name="rows5", bufs=1))
        es_pre = ExitStack()
        ppre = es_pre.enter_context(tc.tile_pool(name="ppre", bufs=1, space="PSUM"))
        stat_rows = []
        for s, src4 in enumerate([rho_x, v_x, u_e, u_x, v_e]):
            row = r5.tile([1, SH], f32, tag=f"strow{s}", name=f"strow{s}")
            for i in range(4):
                pt = ppre.tile([1, 128], f32, tag="ptrow", name="ptrow")
                nc.tensor.transpose(pt[:], src4[i][:], ident[:])
                nc.scalar.copy(out=row[:, i * 128:(i + 1) * 128], in_=pt[:])
            stat_rows.append(row)
            nc.sync.dma_start(agv_in[s:s + 1, :], row[:])
        nc.gpsimd.collective_compute("AllGather", OP.bypass, replica_groups=RG,
                                     ins=[agv_in.ap()], outs=[agv_out.ap()])

        # rho_e partial row via wide matmuls: re[e] = sum_p rho_x[p]*H[p,e]
        rho_b = [sm.tile([128, 1], bf16, tag=f"rhob{i}", name=f"rhob{i}") for i in range(4)]
        for i in range(4):
            nc.vector.tensor_copy(out=rho_b[i][:], in_=rho_x[i][:])
        for nt in range(NJT):
            ps = ppre.tile([1, JT], f32, tag="psre", name="psre")
            for i in range(4):
                nc.tensor.matmul(ps[:], rho_b[i][:],
                                 hf[i][:, nt * JT:(nt + 1) * JT],
                                 start=(i == 0), stop=(i == 3))
            rev = r5.tile([1, JT], f32, tag="rerow", name="rerow", bufs=2)
            nc.scalar.copy(out=rev[:], in_=ps[:])
            nc.sync.dma_start(re_io[0:1, nt * JT:(nt + 1) * JT], rev[:])
        nc.gpsimd.collective_compute("AllReduce", OP.add, replica_groups=RG,
                                     ins=[re_io.ap()], outs=[re_ar.ap()])

        # =================================================================
        # PHASE C5: global maxes; my rho_e shard; attention scales
        # =================================================================
        gmax = {}
        vrowxb = r5.tile([1, N], bf16, tag="vrowxb", name="vrowxb")
        vroweb = r5.tile([1, N], bf16, tag="vroweb", name="vroweb")
        for col, nm in [(0, "mrx"), (1, "mvx"), (2, "mue"), (3, "mux"),
                        (4, "mve")]:
            grow = r5.tile([1, N], f32, tag="grow", name=f"grow{col}", bufs=2)
            for c in range(NCORE):
                nc.sync.dma_start(grow[:, c * SH:(c + 1) * SH],
                                  agv_out[c * 5 + col:c * 5 + col + 1, :])
            mt = sm.tile([1, 1], f32, tag=f"g{nm}", name=f"g{nm}")
            nc.vector.reduce_max(mt[:], grow[:], axis=AX.X)
            gmax[nm] = mt
            if nm == "mvx":
                nc.vector.tensor_copy(out=vrowxb[:], in_=grow[:])
            if nm == "mve":
                nc.vector.tensor_copy(out=vroweb[:], in_=grow[:])
        re_row = r5.tile([1, N], f32, tag="grow", name="rear", bufs=2)
        nc.sync.dma_start(re_row[:], re_ar[0:1, :])
        mre = sm.tile([1, 1], f32, tag="gmre", name="gmre")
        nc.vector.reduce_max(mre[:], re_row[:], axis=AX.X)

        # my 512 rho_e values: one-hot select matmul + transposes
        re8f = sm.tile([8, SH], f32, tag="re8f", name="re8f")
        nc.sync.dma_start(re8f[:],
                          re_ar.ap().rearrange("one (c e) -> (one c) e", c=8))
        re8 = sm.tile([8, SH], bf16, tag="re8", name="re8")
        nc.vector.tensor_copy(out=re8[:], in_=re8f[:])
        myhf = sm.tile([8, 1], f32, tag="myhf", name="myhf")
        nc.sync.dma_start(myhf[:], myhot_in[:, :])
        myh = sm.tile([8, 1], bf16, tag="myh", name="myh")
        nc.vector.tensor_copy(out=myh[:], in_=myhf[:])
        psmy = ppre.tile([1, SH], f32, tag="psmy", name="psmy")
        nc.tensor.matmul(psmy[:], myh[:], re8[:], start=True, stop=True)
        re_my = r5.tile([1, SH], f32, tag="remy", name="remy")
        nc.scalar.copy(out=re_my[:], in_=psmy[:])
        rho_e_sl = [sm.tile([128, 1], f32, tag=f"res{i}", name=f"res{i}") for i in range(4)]
        for i in range(4):
            pt = ppre.tile([128, 1], f32, tag="ptcol", name="ptcol")
            nc.tensor.transpose(pt[:], re_my[:, i * 128:(i + 1) * 128],
                                ident[0:1, 0:1])
            nc.scalar.copy(out=rho_e_sl[i][:], in_=pt[:])

        def lrelu_scalar(nm, a, b):
            s = sm.tile([1, 1], f32, tag=f"ls{nm}", name=f"ls{nm}")
            nc.vector.tensor_tensor(out=s[:], in0=a[:], in1=b[:], op=OP.add)
            s2 = sm.tile([1, 1], f32, tag=f"ls2{nm}", name=f"ls2{nm}")
            nc.vector.tensor_scalar(out=s2[:], in0=s[:], scalar1=SLOPE,
                                    scalar2=None, op0=OP.mult)
            mo = sm.tile([1, 1], f32, tag=f"lm{nm}", name=f"lm{nm}")
            nc.vector.tensor_tensor(out=mo[:], in0=s[:], in1=s2[:], op=OP.max)
            return mo

        def att_scale(nm, maxa, maxr):
            rr = sm.tile([1, 1], f32, tag=f"rr{nm}", name=f"rr{nm}")
            nc.vector.reciprocal(out=rr[:], in_=maxr[:])
            sc = sm.tile([1, 1], f32, tag=f"sc{nm}", name=f"sc{nm}")
            nc.vector.tensor_tensor(out=sc[:], in0=maxa[:], in1=rr[:],
                                    op=OP.mult)
            scb = sm.tile([128, 1], f32, tag=f"scb{nm}", name=f"scb{nm}")
            nc.gpsimd.partition_broadcast(scb[:], sc[:])
            return scb

        max_ax = lrelu_scalar("x", gmax["mux"], gmax["mvx"])
        max_ae = lrelu_scalar("e", gmax["mue"], gmax["mve"])
        scxb = att_scale("x", max_ax, gmax["mrx"])
        sceb = att_scale("e", max_ae, mre)

        def exp_biases(nm, uvec, rvec, scb):
            b1, b2 = [], []
            for i in range(4):
                rt = sm.tile([128, 1], f32, tag=f"rt{nm}{i}", name=f"rt{nm}{i}")
                nc.vector.tensor_scalar(out=rt[:], in0=rvec[i][:],
                                        scalar1=scb[:], scalar2=None,
                                        op0=OP.mult)
                t1 = sm.tile([128, 1], f32, tag=f"b1{nm}{i}", name=f"b1{nm}{i}")
                nc.vector.tensor_tensor(out=t1[:], in0=uvec[i][:], in1=rt[:],
                                        op=OP.add)
                t2 = sm.tile([128, 1], f32, tag=f"b2{nm}{i}", name=f"b2{nm}{i}")
                nc.vector.scalar_tensor_tensor(out=t2[:], in0=uvec[i][:],
                                               scalar=SLOPE, in1=rt[:],
                                               op0=OP.mult, op1=OP.add)
                b1.append(t1)
                b2.append(t2)
            return b1, b2

        b1x, b2x = exp_biases("x", u_x, rho_x, scxb)
        b1e, b2e = exp_biases("e", u_e, rho_e_sl, sceb)
        es_pre.close()

        # =================================================================
        # PHASE C6: exp_x/exp_e (bf16); denom+numer -> packed RS per side
        # =================================================================
        es_ex = ExitStack()
        exp_pool = es_ex.enter_context(tc.tile_pool(name="expp", bufs=4))
        ppX = es_ex.enter_context(tc.tile_pool(name="ppX", bufs=2, space="PSUM"))

        def exp_phase(expt, vrow, b1, b2):
            for jt in range(NJT):
                sl_ = (slice(None), slice(jt * JT, (jt + 1) * JT))
                vch = exp_pool.tile([128, JT], bf16, tag="vch", name="vch",
                                    bufs=2)
                nc.gpsimd.partition_broadcast(vch[:], vrow[0:1, jt * JT:(jt + 1) * JT])
                for i in range(4):
                    e1 = tp.tile([128, JT], bf16, tag="t512", name="e1t", bufs=4)
                    nc.scalar.activation(out=e1[:], in_=vch[:], func=AF.Exp,
                                         bias=b1[i][:], scale=1.0)
                    e2 = tp.tile([128, JT], bf16, tag="t512", name="e2t", bufs=4)
                    nc.scalar.activation(out=e2[:], in_=vch[:], func=AF.Exp,
                                         bias=b2[i][:], scale=SLOPE)
                    nc.vector.tensor_tensor(out=expt[i][sl_], in0=e1[:],
                                            in1=e2[:], op=OP.max)

        def denom_phase(expt, dram_t, pp_dn):
            for nt in range(NJT):
                ps = pp_dn.tile([1, JT], f32, tag="psdn", name="psdn")
                for i in range(4):
                    me = tp.tile([128, JT], bf16, tag="t512", name="metmp", bufs=4)
                    nc.vector.tensor_tensor(
                        out=me[:], in0=expt[i][:, nt * JT:(nt + 1) * JT],
                        in1=hf[i][:, nt * JT:(nt + 1) * JT], op=OP.mult)
                    nc.tensor.matmul(ps[:], ones_b[:], me[:],
                                     start=(i == 0), stop=(i == 3))
                dr = r5.tile([1, JT], bf16, tag="dnrow", name="dnrow", bufs=2)
                nc.scalar.copy(out=dr[:], in_=ps[:])
                nc.sync.dma_start(dram_t[nt, SH:SH + 1, :], dr[:, 0:256])
                nc.sync.dma_start(dram_t[nt, SH + 1:SH + 2, :], dr[:, 256:512])

        def numer_phase(expt, rhs_tiles, dram_t, pp_nm):
            for m in range(32):
                ps = pp_nm.tile([128, D], f32, tag="psnum", name="psnum")
                for i in range(4):
                    nc.tensor.matmul(ps[:], expt[i][:, m * 128:(m + 1) * 128],
                                     rhs_tiles[i][:], start=(i == 0),
                                     stop=(i == 3))
                ev = tp.tile([128, D], bf16, tag="t256b", name="numev", bufs=6)
                nc.scalar.copy(out=ev[:], in_=ps[:])
                nc.sync.dma_start(
                    dram_t[m // 4, (m % 4) * 128:(m % 4) * 128 + 128, :], ev[:])

        exp_x = [exp_pool.tile([128, N], bf16, tag="expx", name="expx") for _ in range(4)]
        exp_phase(exp_x, vrowxb, b1x, b2x)
        denom_phase(exp_x, nxd_in, ppX)
        numer_phase(exp_x, xlw, nxd_in, ppX)
        # exp_e reuses the exp_x buffers (ring of 4; WAR on numer_x reads).
        # Emitted before the RS so its gpsimd broadcasts don't queue behind
        # the collective; the RS launches as soon as its DMA deps complete
        # and exp_e/denom_e overlap it.
        exp_e = [exp_pool.tile([128, N], bf16, tag="expx", name="expe") for _ in range(4)]
        exp_phase(exp_e, vroweb, b1e, b2e)
        denom_phase(exp_e, ned_in, ppX)
        nc.gpsimd.collective_compute("ReduceScatter", OP.add, replica_groups=RG,
                                     ins=[nxd_in.ap()], outs=[nxd_rs.ap()])

        # =================================================================
        # PHASE C7: E_tilde shard = elu(numer/denom); numer_e; packed RS
        # =================================================================
        def elu_tile(out_ap, in_ap, rscale):
            r0 = tp.tile([128, D], f32, tag="t256", name="elur", bufs=6)
            nc.scalar.activation(out=r0[:], in_=in_ap, func=AF.Relu,
                                 scale=rscale)
            m0 = tp.tile([128, D], f32, tag="t256", name="elum", bufs=6)
            nc.vector.tensor_scalar(out=m0[:], in0=in_ap, scalar1=rscale,
                                    scalar2=0.0, op0=OP.mult, op1=OP.min)
            e0 = tp.tile([128, D], f32, tag="t256", name="elue", bufs=6)
            nc.scalar.activation(out=e0[:], in_=m0[:], func=AF.Exp)
            nc.vector.scalar_tensor_tensor(out=out_ap, in0=r0[:], scalar=-1.0,
                                           in1=e0[:], op0=OP.add, op1=OP.add)

        def recip_cols(dram_t, nm):
            # rows [SH..SH+2) of dram_t hold the 512 denominators; return
            # per-partition reciprocal tiles [128,1] x4
            drow = r5.tile([1, JT], f32, tag="drr", name=f"dr{nm}")
            dtmp = r5.tile([1, JT], bf16, tag="dbr", name=f"db{nm}")
            nc.sync.dma_start(dtmp[:, 0:256], dram_t[SH:SH + 1, :])
            nc.sync.dma_start(dtmp[:, 256:512], dram_t[SH + 1:SH + 2, :])
            nc.vector.tensor_copy(out=drow[:], in_=dtmp[:])
            rrow = r5.tile([1, JT], f32, tag="rrr", name=f"rr{nm}")
            nc.vector.reciprocal(out=rrow[:], in_=drow[:])
            outs = []
            for i in range(4):
                pt = ppX.tile([128, 1], f32, tag="ptc2", name="ptc2")
                nc.tensor.transpose(pt[:], rrow[:, i * 128:(i + 1) * 128],
                                    ident[0:1, 0:1])
                rc = sm.tile([128, 1], f32, tag=f"rc{nm}{i}", name=f"rc{nm}{i}")
                nc.scalar.copy(out=rc[:], in_=pt[:])
                outs.append(rc)
            return outs

        rdx = recip_cols(nxd_rs, "x")
        etil = [cpl.tile([128, D], bf16, tag=f"et{i}", name=f"et{i}") for i in range(4)]
        for i in range(4):
            nxs = tp.tile([128, D], bf16, tag="t256b", name="nxs", bufs=6)
            nc.sync.dma_start(nxs[:], nxd_rs[i * 128:(i + 1) * 128, :])
            nxf = tp.tile([128, D], f32, tag="t256", name="nxf", bufs=6)
            nc.vector.tensor_copy(out=nxf[:], in_=nxs[:])
            elu_tile(etil[i][:], nxf[:], rdx[i][:])

        numer_phase(exp_e, etil, ned_in, ppX)
        nc.gpsimd.collective_compute("ReduceScatter", OP.add, replica_groups=RG,
                                     ins=[ned_in.ap()], outs=[ned_rs.ap()])

        # =================================================================
        # PHASE C8: my X_tilde shard = elu(ne/dne) -> out
        # =================================================================
        rde = recip_cols(ned_rs, "e")
        for i in range(4):
            nes = tp.tile([128, D], bf16, tag="t256b", name="nes", bufs=6)
            nc.sync.dma_start(nes[:], ned_rs[i * 128:(i + 1) * 128, :])
            nef = tp.tile([128, D], f32, tag="t256", name="nef", bufs=6)
            nc.vector.tensor_copy(out=nef[:], in_=nes[:])
            ot = tp.tile([128, D], f32, tag="t256", name="otile", bufs=6)
            elu_tile(ot[:], nef[:], rde[i][:])
            nc.sync.dma_start(out_t[i * 128:(i + 1) * 128, :], ot[:])

        es_ex.close()
        es_c4.close()
        es_c.close()
        es_hf.close()

    nc.compile()
    return nc


_CACHE = {}


def _get_prog():
    if "nc" not in _CACHE:
        _CACHE["nc"] = _build()
    return _CACHE["nc"]


def make_in_maps(X, theta, W, alpha_x, alpha_e):
    import ml_dtypes
    bf = ml_dtypes.bfloat16

    def hi_lo(a):
        hi = a.astype(bf)
        lo = (a - hi.astype(np.float32)).astype(bf)
        return hi, lo

    X = np.ascontiguousarray(np.asarray(X, np.float32))
    theta = np.ascontiguousarray(np.asarray(theta, np.float32))
    W = np.ascontiguousarray(np.asarray(W, np.float32))
    alpha_x = np.asarray(alpha_x, np.float32).reshape(-1)
    alpha_e = np.asarray(alpha_e, np.float32).reshape(-1)
    XT = np.ascontiguousarray(X.T)                       # [784, 4096]
    xt7 = np.ascontiguousarray(XT.reshape(NKF, KCH, N))
    xt7h, xt7l = hi_lo(xt7)
    xt7h, xt7l = np.ascontiguousarray(xt7h), np.ascontiguousarray(xt7l)
    nsq = (-0.5 * np.sum(X.astype(np.float64) ** 2, axis=1)) \
        .astype(np.float32).reshape(1, N)
    nsqh, nsql = hi_lo(nsq)
    nsqh, nsql = np.ascontiguousarray(nsqh), np.ascontiguousarray(nsql)
    th7 = np.ascontiguousarray(theta.reshape(NKF, KCH, D).astype(bf))
    w2 = np.ascontiguousarray(W.reshape(2, 128, D).astype(bf))
    wv = np.ascontiguousarray(np.stack([
        (W.astype(np.float64) @ alpha_x[D:].astype(np.float64)).astype(np.float32),
        (W.astype(np.float64) @ alpha_e[:D].astype(np.float64)).astype(np.float32),
    ]))
    al = np.ascontiguousarray(np.stack([alpha_x, alpha_e]))  # [2, 512]
    in_maps = []
    for c in range(NCORE):
        csl = slice(c * SH, (c + 1) * SH)
        xtch = np.ascontiguousarray(xt7h[:, :, csl])
        xtcl = np.ascontiguousarray(xt7l[:, :, csl])
        myhot = np.zeros((NCORE, 1), np.float32)
        myhot[c, 0] = 1.0
        in_maps.append({"xth": xt7h, "xtl": xt7l, "nsqh": nsqh, "nsql": nsql,
                        "xtch": xtch, "xtcl": xtcl, "theta": th7,
                        "w": w2, "wv": wv, "alpha": al, "myhot": myhot})
    return in_maps


def kernel(X, theta, W, alpha_x, alpha_e):
    nc = _get_prog()
    in_maps = make_in_maps(X, theta, W, alpha_x, alpha_e)
    res = run_bass_kernel_spmd(nc, in_maps, core_ids=list(range(NCORE)))
    return np.concatenate(
        [np.asarray(res.results[c]["out"], np.float32) for c in range(NCORE)],
        axis=0)
